# revision 52
# baseline (speedup 1.0000x reference)
"""Trainium2 Bass kernel for nn_BaseNCA (3x3 Sobel NCA + per-pixel MLP, 4 steps).

Sharding: pure data parallel over 8 cores = (batch b, H-half). Each core gets
one batch's top or bottom half of H (128 rows) plus a 4-row halo; the bottom
half is row-FLIPPED host-side (ky is antisymmetric, kx symmetric under row
flip, so only the ky sign folds) so every core's valid region shrinks at the
same (bottom) edge and one SPMD graph serves all cores. Validity shrinks one
row per step; step s computes rows 0..130-s, so no collectives are needed.

Device layout: state master (fp32, ping-pong A/B) and an fp8(e4m3) mirror,
[128 partitions = (ch + 16*(col%8)), free = (row, t=col//8)] with one zero pad
column each side (row stride 34) and a zero guard row above/below. All matmuls
run as fp8 DoubleRow (0.5 cycles/row): the DR "pair" dim is a separate AP dim
with arbitrary stride, so
  - fc1: per class 2 DRs: A pairs row-shifts (di=-1, di=0), B covers di=+1
    (pair slot zeroed); the W-wraparound classes 0/7 use B's spare slot plus
    one extra DR for the +-1 t-offset neighbor-class taps.
  - fc2: even/odd row-pair DRs (same lhsT, shifted rhs base, negative pair
    stride for odd so reads stay in bounds).
  - fc3: class-pair DRs -- h2 for all 8 classes sits in one tile with class a
    free dim, and each DR's two pair slots carry w3 embedded at the two
    classes' output partitions, accumulating one PSUM bank of dx for all 128
    (class, ch) partitions.
Scales: Keff*S1, w2*S2, w3*S3 (powers of 2, data-calibrated in make_in_maps);
stored h1 = S1*relu(.), stored h2 = S1*S2*relu(. + S1*S2*b2) -- the same
formula is exact on both the Activation engine (Relu with bias ptr) and DVE
(tensor_scalar add+max), so h1/h2 units route freely between ACT and DVE for
load balance (the two engines are the bottleneck; PE has ~45% slack). The
state update is one DVE scalar_tensor_tensor, dst = ps3/(S1*S2*S3) + src --
calibration shows |dx| stays ~80x below the +-1 clip, so the clip is dropped
(use_clip=True rebuilds the guarded 3-op path if an input ever needs it). The
fp8 mirror refresh runs on the otherwise-idle gpsimd engine.

Row blocks are sized evenly (14-16 rows) so no undersized remainder block
starves the pipeline at step boundaries; the h1 ring buffers' last row is
pre-zeroed since odd-height blocks pair it into fc2's dead DoubleRow slot
(uninitialized fp8 bytes can be NaN, and 0*NaN would poison the PSUM).
Emission is a flat software pipeline over (step, block, class-pair) slots:
fc3+tail at slot k-6, fc2+h2 at k-2, fc1+h1 at k, so every engine's in-order
queue always holds ready work; one 2-bank PSUM tile per slot serves ps1 then
ps2 (3-deep ring), ps3 accumulates class-pair DRs in its own bank. A few
state updates per step run as ACT scale-copy + gpsimd add to balance DVE.
Startup: the ACT function table is pre-loaded ahead of the serial input-DMA
chain, and dummy fp8 matmuls on a zeroed tile fill the DMA wait so the PE
p-state is fully ramped when real work arrives. Cost-model timeline:
~185.7us vs the 377.8us f32r baseline (~2.03x), within ~12% of the ACT+DVE
busy-time floor (only ACT and DVE can read PSUM, so the two MLP layers'
activations bound the kernel at ~1 elem/cycle/engine).
"""

import sys

import numpy as np

sys.path.insert(0, "/opt/trn_rl_repo")

import ml_dtypes
import concourse.bass as bass
import concourse.mybir as mybir
from concourse.bacc import Bacc
from concourse.bass_utils import run_bass_kernel_spmd
from concourse.tile import TileContext

C, HID, W = 16, 128, 256
T = W // 8          # 32 t-slots per class
RS = T + 2          # row stride incl. one pad col each side
HE = 132            # extended rows per core (128 kept + 4 halo)
NR_TOT = 1 + HE + 1  # incl. zero guard rows
SX = np.array([[-1.0, 0.0, 1.0], [-2.0, 0.0, 2.0], [-1.0, 0.0, 1.0]], np.float64)
SY = SX.T

f32 = mybir.dt.float32
f8 = mybir.dt.float8e4
DR = mybir.MatmulPerfMode.DoubleRow
npf8 = ml_dtypes.float8_e4m3
ADD, MAX, MIN, MULT = (mybir.AluOpType.add, mybir.AluOpType.max,
                       mybir.AluOpType.min, mybir.AluOpType.mult)

# scales (powers of two), validated against calibration in make_in_maps
S1, S2, S3 = 16.0, 2.0, 256.0
SSTAR = S1 * S2 * S3

# lhsT kind order in the w1 tensor: A0,B0,A1,B1,...,A7,B7,W0,W7
N_KINDS = 18


def blocks_for_step(s):
    """Row blocks for step s (computes rows 0..130-s), sized evenly so no
    undersized remainder block starves the engine pipeline."""
    nrows = 131 - s
    nb = (nrows + 15) // 16
    base, ext = divmod(nrows, nb)
    out, i = [], 0
    for b in range(nb):
        nr = base + (1 if b < ext else 0)
        out.append((i, nr))
        i += nr
    return out


def set_ap(apv, dims, off_delta=0):
    """Copy of AP with free dims replaced by [(stride, count), ...]."""
    b = apv.copy()
    while b.ndim - 1 < len(dims):
        b = b.unsqueeze(1)
    raw = b.ap
    assert len(raw) == len(dims) + 1
    for i, (st, cn) in enumerate(dims):
        raw[i + 1] = (st, cn)
    b.ap = raw
    b.offset = b.offset + off_delta
    return b


def build_graph(nc, n_steps, act_share=(11, 20), sk1=2, sk2=4, order="321",
                use_clip=False, pipe_at=(1, 6), n_warm=8, interleave=False):
    relu = mybir.ActivationFunctionType.Relu

    xbin = nc.declare_dram_parameter("xb", [128, NR_TOT, RS], f8, isOutput=False)
    xmin = nc.declare_dram_parameter("xm", [128, NR_TOT, RS], f32, isOutput=False)
    w1in = nc.declare_dram_parameter("w1", [128, N_KINDS, 2, 128], f8, isOutput=False)
    w2in = nc.declare_dram_parameter("w2", [128, 2, 128], f8, isOutput=False)
    w3in = nc.declare_dram_parameter("w3", [128, 4, 2, 128], f8, isOutput=False)
    thrin = nc.declare_dram_parameter("thr", [128, 2], f32, isOutput=False)
    outp = nc.declare_dram_parameter("out", [128, 128, RS], f32, isOutput=True)

    with TileContext(nc) as tc:
        with (
            tc.tile_pool(name="const", bufs=1) as cpool,
            tc.tile_pool(name="h1p", bufs=6) as h1pool,
            tc.tile_pool(name="h2p", bufs=4) as h2pool,
            tc.tile_pool(name="up", bufs=4) as upool,
            tc.tile_pool(name="P", bufs=3, space="PSUM") as Ppool,  # shared ps1/ps2
            tc.tile_pool(name="P3", bufs=2, space="PSUM") as P3pool,
        ):
            stA = cpool.tile([128, NR_TOT, RS], f32, tag="stA")
            stB = cpool.tile([128, NR_TOT, RS], f32, tag="stB")
            mir = cpool.tile([128, NR_TOT, RS], f8, tag="mir")
            w1 = cpool.tile([128, N_KINDS, 2, 128], f8, tag="w1")
            w2 = cpool.tile([128, 2, 128], f8, tag="w2")
            w3 = cpool.tile([128, 4, 2, 128], f8, tag="w3")
            thr = cpool.tile([128, 2], f32, tag="thr")

            # pre-zero the h1t ring buffers' last row: odd-nr blocks pair it
            # into fc2's dead DR slot, and uninitialized fp8 bytes can be NaN
            # (0 * NaN = NaN would poison the accumulation)
            for _ in range(6):
                hz = h1pool.tile([128, 2, 16, T], f8, tag="h1t", name="h1t")
                nc.gpsimd.memset(hz[:, :, 15, :], 0.0)

            # force the ACT function-table load ahead of the input DMAs
            warm = cpool.tile([128, 1], f32, tag="warm")
            nc.scalar.activation(warm[:, :], warm[:, :], relu, bias=0.0,
                                 scale=0.0)
            # PE p-state warm-up: dummy DR matmuls on an uninitialized tile
            # fill the input-DMA wait so real matmuls start fully ramped
            garb = cpool.tile([128, 1100], f8, tag="garb")
            nc.gpsimd.memset(garb[:, :], 0.0)
            dmy = Ppool.tile([128, 2, 16, T], f32, tag="ps", name="ps")
            dl = set_ap(garb[:, 0], [(128, 2), (1, 128)])
            dr = set_ap(garb[:, 0], [(512, 2), (1, 512)])
            for _ in range(n_warm):
                nc.tensor.matmul(dmy[:, 0, :, :], dl, dr, start=True,
                                 stop=True, perf_mode=DR)
            # load order matters: the DMA device is serial, so ship exactly
            # what block-0 compute needs first (mirror rows 0..39 + weights),
            # then stream the rest under compute
            nc.sync.dma_start(out=mir[:, 0:20, :], in_=xbin[:, 0:20, :])
            nc.sync.dma_start(out=w1[:, 0:8, :, :], in_=w1in[:, 0:8, :, :])
            nc.sync.dma_start(out=w1[:, 16:18, :, :], in_=w1in[:, 16:18, :, :])
            nc.sync.dma_start(out=thr[:, :], in_=thrin[:, :])
            nc.sync.dma_start(out=mir[:, 20:60, :], in_=xbin[:, 20:60, :])
            nc.sync.dma_start(out=w2[:, :, :], in_=w2in[:, :, :])
            nc.sync.dma_start(out=w1[:, 8:16, :, :], in_=w1in[:, 8:16, :, :])
            nc.sync.dma_start(out=w3[:, :, :, :], in_=w3in[:, :, :, :])
            nc.sync.dma_start(out=mir[:, 60:NR_TOT, :], in_=xbin[:, 60:NR_TOT, :])
            for c in range(4):
                r0, r1 = c * 34, min((c + 1) * 34, NR_TOT)
                nc.sync.dma_start(out=stA[:, r0:r1, :], in_=xmin[:, r0:r1, :])

            unit = 0  # h-unit counter for ACT/DVE routing
            tctr = [0]  # tail counter for dst routing
            PIPE_AT = pipe_at
            acc = [0.0, 0.0]  # greedy routing: accumulated ACT / DVE ns

            def h_op(out_ap, in_ap, bias_ap):
                nonlocal unit
                if act_share == "greedy":
                    n = in_ap.free_size()
                    ca, cd = n / 1.2 + 185, n / 0.96 + 125
                    on_act = acc[0] + ca <= acc[1] + cd
                    acc[0 if on_act else 1] += ca if on_act else cd
                else:
                    ph = act_share[2] if len(act_share) > 2 else 0
                    on_act = ((unit + ph) * act_share[0]) % act_share[1] \
                        < act_share[0]
                unit += 1
                if on_act:
                    nc.scalar.activation(out_ap, in_ap, relu, bias=bias_ap,
                                         scale=1.0)
                else:
                    nc.vector.tensor_scalar(out_ap, in_ap, bias_ap, 0.0, ADD, MAX)

            # flat global pipeline over (step, block, pair) slots with skew:
            # fc1 at slot k, fc2 at k-SK1, fc3 at k-SK1-SK2 -- so the PE
            # stream always has ready work while h1/h2 ops run on ACT/DVE.
            SK1, SK2 = sk1, sk2
            slots = []
            for s in range(n_steps):
                bl = blocks_for_step(s)
                if interleave:
                    bi = 0
                    while bi < len(bl):
                        grp = bl[bi:bi + 2]
                        for p in range(4):
                            for i0, nr in grp:
                                slots.append((s, i0, nr, p))
                        bi += 2
                else:
                    for i0, nr in bl:
                        for p in range(4):
                            slots.append((s, i0, nr, p))
            n_sl = len(slots)
            psd, h1d = {}, {}   # slot idx -> tiles
            h2d, ps3d = {}, {}  # (s, i0) -> tiles

            def st_pair(s):
                return (stA, stB) if s % 2 == 0 else (stB, stA)

            def do_fc1(k):
                s, i0, nr, p = slots[k]
                pst = Ppool.tile([128, 2, 16, T], f32, tag="ps", name="ps")
                psd[k] = pst
                for j in (0, 1):
                    cls = 2 * p + j
                    edge = cls in (0, 7)
                    # A: pair slots = state rows (i-1, i)
                    rhs = set_ap(mir[:, 0, 0], [(RS, 2), (RS, nr), (1, T)],
                                 i0 * RS + 1)
                    nc.tensor.matmul(pst[:, j, :nr, :], w1[:, 2 * cls, :, :],
                                     rhs, start=True, stop=False, perf_mode=DR)
                    # B: slot0 = row i+1; slot1: cls0 -> (i+1, t-1),
                    # cls7 -> (i+1, t+1), interior -> dead (lhsT 0)
                    d1 = -1 if cls == 0 else (1 if cls == 7 else -RS)
                    rhs = set_ap(mir[:, 0, 0], [(d1, 2), (RS, nr), (1, T)],
                                 (i0 + 2) * RS + 1)
                    nc.tensor.matmul(pst[:, j, :nr, :],
                                     w1[:, 2 * cls + 1, :, :], rhs,
                                     start=False, stop=not edge, perf_mode=DR)
                    if edge:
                        co = 0 if cls == 0 else 2
                        rhs = set_ap(mir[:, 0, 0], [(RS, 2), (RS, nr), (1, T)],
                                     i0 * RS + co)
                        nc.tensor.matmul(pst[:, j, :nr, :],
                                         w1[:, 16 + (cls == 7), :, :], rhs,
                                         start=False, stop=True, perf_mode=DR)
                h1t = h1pool.tile([128, 2, 16, T], f8, tag="h1t", name="h1t")
                h1d[k] = h1t
                h_op(h1t[:, :, :nr, :], pst[:, :, :nr, :], thr[:, 0:1])

            def do_fc2(k):
                s, i0, nr, p = slots[k]
                if p == 0:
                    h2d[(s, i0)] = h2pool.tile([128, 8, 16, T], f8, tag="h2t",
                                               name="h2t")
                h2t = h2d[(s, i0)]
                pst, h1t = psd[k], h1d.pop(k)
                ne, no = (nr + 1) // 2, nr // 2
                p2b = pst[:, 0, 0, 0]
                for j in (0, 1):
                    base = h1t[:, j, 0, 0]
                    # even/odd rows write interleaved into ps2 (natural row
                    # order) so every AP here and below stays <= 3 free dims
                    rhs = set_ap(base, [(T, 2), (2 * T, ne), (1, T)])
                    nc.tensor.matmul(
                        set_ap(p2b, [(2 * T, ne), (1, T)], j * 512),
                        w2[:, :, :], rhs,
                        start=True, stop=no == 0, perf_mode=DR)
                    if no:
                        rhs = set_ap(base, [(-T, 2), (2 * T, no), (1, T)], T)
                        nc.tensor.matmul(
                            set_ap(p2b, [(2 * T, no), (1, T)], j * 512 + T),
                            w2[:, :, :], rhs,
                            start=False, stop=True, perf_mode=DR)
                # h2 out: classes 2p,2p+1, rows already in natural order
                i_ = set_ap(p2b, [(512, 2), (T, nr), (1, T)])
                h_op(h2t[:, 2 * p:2 * p + 2, :nr, :], i_, thr[:, 1:2])
                psd.pop(k)

            def do_fc3(k):
                s, i0, nr, p = slots[k]
                if p == 0:
                    ps3d[(s, i0)] = P3pool.tile([128, 16, T], f32, tag="ps3",
                                                name="ps3")
                ps3 = ps3d[(s, i0)]
                h2t = h2d[(s, i0)]
                nc.tensor.matmul(ps3[:, :nr, :], w3[:, p, :, :],
                                 h2t[:, 2 * p:2 * p + 2, :nr, :],
                                 start=p == 0, stop=p == 3, perf_mode=DR)
                if p != 3:
                    return
                # block tail: state update, mirror refresh / output
                src, dst = st_pair(s)
                ps3d.pop((s, i0))
                if use_clip:
                    ut = upool.tile([128, 16, T], f32, tag="ut")
                    nc.vector.tensor_scalar(ut[:, :nr, :], ps3[:, :nr, :],
                                            -SSTAR, 1.0 / SSTAR, MAX, MULT)
                    us = upool.tile([128, 16, T], f32, tag="us")
                    nc.gpsimd.tensor_scalar(us[:, :nr, :], ut[:, :nr, :],
                                            1.0, None, MIN)
                    nc.gpsimd.tensor_tensor(
                        dst[:, 1 + i0:1 + i0 + nr, 1:1 + T], us[:, :nr, :],
                        src[:, 1 + i0:1 + i0 + nr, 1:1 + T], ADD)
                elif (acc[1] + 658 > acc[0] + 612 + 300
                      if act_share == "greedy" else tctr[0] % 9 in PIPE_AT):
                    # relieve DVE: scale-copy on ACT, add on gpsimd
                    tt = upool.tile([128, 16, T], f32, tag="ut")
                    nc.scalar.activation(tt[:, :nr, :], ps3[:, :nr, :],
                                         mybir.ActivationFunctionType.Copy,
                                         bias=0.0, scale=1.0 / SSTAR)
                    nc.gpsimd.tensor_tensor(
                        dst[:, 1 + i0:1 + i0 + nr, 1:1 + T], tt[:, :nr, :],
                        src[:, 1 + i0:1 + i0 + nr, 1:1 + T], ADD)
                    if act_share == "greedy":
                        acc[0] += nr * T / 1.2 + 185
                else:
                    # |dx| never approaches the +-1 clip (calibrated, ~80x
                    # margin): dst = ps3/SSTAR + src in one DVE op
                    nc.vector.scalar_tensor_tensor(
                        dst[:, 1 + i0:1 + i0 + nr, 1:1 + T], ps3[:, :nr, :],
                        1.0 / SSTAR, src[:, 1 + i0:1 + i0 + nr, 1:1 + T],
                        MULT, ADD)
                    if act_share == "greedy":
                        acc[1] += nr * T / 0.96 + 125
                tctr[0] += 1
                if s < n_steps - 1:
                    nc.gpsimd.tensor_copy(
                        mir[:, 1 + i0:1 + i0 + nr, 1:1 + T],
                        dst[:, 1 + i0:1 + i0 + nr, 1:1 + T])
                elif i0 < 128:
                    nrr = min(nr, 128 - i0)
                    nc.sync.dma_start(out=outp[:, i0:i0 + nrr, :],
                                      in_=dst[:, 1 + i0:1 + i0 + nrr, :])

            for k in range(n_sl + SK1 + SK2):
                for ch in order:
                    if ch == "1" and k < n_sl:
                        do_fc1(k)
                    elif ch == "2" and 0 <= k - SK1 < n_sl:
                        do_fc2(k - SK1)
                    elif ch == "3" and 0 <= k - SK1 - SK2 < n_sl:
                        do_fc3(k - SK1 - SK2)
    return nc


# ---------------------------------------------------------------- host side

def fold_weights(gamma, beta, fc1_w, fc1_b, fc2_w, fc2_b, fc3_w, fc3_b, flip):
    """Per-(batch, half) folded weights in float64 (unquantized)."""
    a = np.abs(gamma)
    sg = np.sign(gamma)
    sy = -SY if flip else SY
    W1x, W1gx, W1gy = fc1_w[0:16], fc1_w[16:32], fc1_w[32:48]

    def keff(di, dj):
        k = SX[di + 1, dj + 1] * W1gx + sy[di + 1, dj + 1] * W1gy
        if di == 0 and dj == 0:
            k = k + W1x
        return k * a[None, :]

    b1 = a * fc1_b
    w2e = sg[:, None] * fc2_w
    b2e = beta @ fc2_w + fc2_b
    w3e = 0.1 * fc3_w
    b3e = 0.1 * fc3_b
    return keff, b1, w2e, b2e, w3e, b3e


def build_core_map(keff, b1, w2e, b2e, w3e):
    """Quantized device tensors for one core."""
    w1 = np.zeros((N_KINDS, 2, 128, 128), np.float64)  # [kind, slot, K, M]
    for cls in range(8):
        A, B = 2 * cls, 2 * cls + 1
        for slot, di in ((0, -1), (1, 0)):
            for dj in (-1, 0, 1):
                src = cls + dj
                if 0 <= src <= 7:
                    w1[A, slot, 16 * src:16 * src + 16, :] = keff(di, dj)
        for dj in (-1, 0, 1):
            src = cls + dj
            if 0 <= src <= 7:
                w1[B, 0, 16 * src:16 * src + 16, :] = keff(1, dj)
        if cls == 0:
            w1[B, 1, 16 * 7:, :] = keff(1, -1)
        elif cls == 7:
            w1[B, 1, 0:16, :] = keff(1, 1)
    for slot, di in ((0, -1), (1, 0)):
        w1[16, slot, 16 * 7:, :] = keff(di, -1)  # W0: class 7 at t-1
        w1[17, slot, 0:16, :] = keff(di, 1)      # W7: class 0 at t+1
    w3 = np.zeros((4, 2, 128, 128), np.float64)
    for p in range(4):
        for j in (0, 1):
            cls = 2 * p + j
            w3[p, j, :, 16 * cls:16 * cls + 16] = w3e
    w2q = np.zeros((128, 2, 128), np.float64)
    w2q[:, 0, :] = S2 * w2e
    return {
        "w1": np.ascontiguousarray(
            (S1 * w1).transpose(2, 0, 1, 3)).astype(npf8),
        "w2": w2q.astype(npf8),
        "w3": np.ascontiguousarray((S3 * w3).transpose(2, 0, 1, 3)).astype(npf8),
        "thr": np.stack([S1 * b1, S1 * S2 * b2e], axis=1).astype(np.float32),
    }


def shuffle_in(x_ext):
    """[16, HE, 256] -> [128, NR_TOT, RS] blocked layout with pads/guards."""
    xb = np.zeros((8, 16, NR_TOT, RS), np.float32)
    for r in range(8):
        xb[r, :, 1:1 + HE, 1:1 + T] = x_ext[:, :, r::8]
    return xb.reshape(128, NR_TOT, RS)


def unshuffle_out(res):
    """[128, 128, RS] -> [16, 128, 256]."""
    rb = np.asarray(res, np.float32).reshape(8, 16, 128, RS)
    y = np.empty((16, 128, W), np.float32)
    for r in range(8):
        y[:, :, r::8] = rb[r, :, :, 1:1 + T]
    return y


def calibrate(x, folds, n_steps):
    """Strip-simulate to size fp8 scales and check the clip margin."""
    keff, b1, w2e, b2e, w3e, b3e = folds
    kmax = max(np.abs(keff(di, dj)).max() for di in (-1, 0, 1)
               for dj in (-1, 0, 1))
    st = np.asarray(x[0, :, 48:48 + 24 + 2 * n_steps, :], np.float64)
    h1m = h2m = dxm = 1e-9
    for _ in range(n_steps):
        R = st.shape[1]
        pad = np.zeros((16, R + 2, 258))
        pad[:, 1:R + 1, 1:257] = st
        pre = np.zeros((128, R - 2, 256))
        for di in (-1, 0, 1):
            for dj in (-1, 0, 1):
                pre += np.einsum("chw,cm->mhw",
                                 pad[:, 2 + di:R + di, 1 + dj:257 + dj],
                                 keff(di, dj))
        h1 = np.maximum(pre + b1[:, None, None], 0.0)
        h1m = max(h1m, h1.max())
        h2 = np.maximum(np.einsum("mhw,mn->nhw", h1, w2e)
                        + b2e[:, None, None], 0.0)
        h2m = max(h2m, h2.max())
        dx = np.einsum("nhw,nc->chw", h2, w3e)
        dxm = max(dxm, np.abs(dx).max())
        st = st[:, 1:R - 1, :] + np.clip(dx, -1.0, 1.0)
    return kmax, h1m, h2m, dxm


def pick_scales(kmax, h1m, h2m, w2max, w3max, dxm):
    """Powers of two keeping every fp8-stored magnitude under ~176 (of 240).
    The strip calibration can undershoot global maxima a bit, hence margin."""
    global S1, S2, S3, SSTAR
    lim = 176.0
    S1 = 2.0 ** int(np.floor(np.log2(min(lim / kmax, lim / h1m))))
    S2 = 2.0 ** int(np.floor(np.log2(min(lim / w2max, lim / (S1 * h2m)))))
    S3 = 2.0 ** int(np.floor(np.log2(lim / w3max)))
    SSTAR = S1 * S2 * S3
    return dxm > 0.25


def make_in_maps(inputs):
    x = np.asarray(inputs["x"], np.float32)
    n_steps = int(np.asarray(inputs["n_steps"]))
    cond = np.asarray(inputs["cond"]).astype(np.int64)
    embed = np.asarray(inputs["embed"], np.float64)
    film_w = np.asarray(inputs["film_w"], np.float64)
    film_b = np.asarray(inputs["film_b"], np.float64)
    args = [np.asarray(inputs[k], np.float64) for k in
            ("fc1_w", "fc1_b", "fc2_w", "fc2_b", "fc3_w", "fc3_b")]
    film = embed[cond] @ film_w + film_b
    gamma, beta = film[:, :HID], film[:, HID:]

    all_folds = [fold_weights(gamma[k // 2], beta[k // 2], *args,
                              flip=k % 2 == 1) for k in range(8)]
    kmax = h1m = h2m = dxm = w2max = w3max = 1e-9
    for k in (0, 1):  # one per flip parity; gammas are comparable
        km, h1_, h2_, dx_ = calibrate(x, all_folds[k], n_steps)
        kmax, h1m = max(kmax, km), max(h1m, h1_)
        h2m, dxm = max(h2m, h2_), max(dxm, dx_)
    for f in all_folds:
        kmax = max(kmax, max(np.abs(f[0](di, dj)).max()
                             for di in (-1, 0, 1) for dj in (-1, 0, 1)))
        w2max = max(w2max, np.abs(f[2]).max())
        w3max = max(w3max, np.abs(f[4]).max())
    use_clip = pick_scales(kmax, 1.5 * h1m, 1.5 * h2m, w2max, w3max, dxm)

    in_maps = []
    for k in range(8):
        b, half = k // 2, k % 2
        m = build_core_map(*all_folds[k][:5])
        x_ext = x[b, :, 0:HE, :] if half == 0 else x[b, :, W - HE:W, :][:, ::-1, :]
        xm = shuffle_in(np.ascontiguousarray(x_ext))
        m["xm"] = xm
        m["xb"] = xm.astype(npf8)
        in_maps.append(m)
    return in_maps, use_clip


def assemble_output(results, like):
    y = np.empty_like(like)
    for k in range(8):
        out = unshuffle_out(results[k]["out"])
        b, half = k // 2, k % 2
        if half == 0:
            y[b, :, 0:128, :] = out
        else:
            y[b, :, 128:256, :] = out[:, ::-1, :]
    return y


def kernel(**inputs):
    n_steps = int(np.asarray(inputs["n_steps"]))
    x = np.asarray(inputs["x"], np.float32)
    in_maps, use_clip = make_in_maps(inputs)
    nc = Bacc()
    build_graph(nc, n_steps, use_clip=use_clip)
    nc.finalize()
    res = run_bass_kernel_spmd(nc, in_maps, core_ids=list(range(8)))
    return assemble_output(res.results, x)
